# revision 1
# baseline (speedup 1.0000x reference)
"""MixUp1D on Trainium2 (Bass/Tile), 8-core data-parallel.

Computes, for x:(N,C,L), y:(N,NC), perm/mask/lam:(N,):
    w        = where(mask, lam, 1.0)                  (host, O(N) scalars)
    aug_x[i] = w[i]*x[i] + (1-w[i])*x[perm[i]]
    aug_y[i] = w[i]*y[i] + (1-w[i])*y[perm[i]]

Sharding: N is split across 8 cores (32 samples each). perm indexes the
whole batch, so the cross-shard gather x[perm] is materialized on the
host while slicing the per-core inputs (the "all-to-all" realized at
shard time); each core then streams its 32 (x, x[perm]) sample pairs
and blends them with per-partition scalar weights.

Per-core device layout: the 32-sample shard (32 x 262144 f32) is viewed
as (2048, 4096); each [128, 4096] tile is one contiguous 2 MB DMA and
holds exactly 2 samples (64 partitions each), so the per-sample weight
becomes a per-partition scalar:
    tw = ACT:  w * x            (activation Copy with per-partition scale)
    o  = DVE:  (xp * (1-w)) + tw  (fused scalar_tensor_tensor)
"""

import numpy as np

import concourse.bacc as bacc
import concourse.mybir as mybir
from concourse.tile import TileContext
from concourse.bass_utils import run_bass_kernel_spmd

# Problem shapes (fixed for this problem)
N, C, L = 256, 16, 16384
NCLS = 8
NCORES = 8
SHARD = N // NCORES          # 32 samples per core
ELEM = C * L                 # 262144 elements per sample
F = 4096                     # SBUF tile free dim
ROWS = SHARD * ELEM // F     # 2048 rows of F per core
P = 128                      # SBUF partitions
NT = ROWS // P               # 16 x-tiles per core
SPT = (P * F) // ELEM        # samples per tile (2)
PPS = P // SPT               # partitions per sample (64)

_CACHE: dict = {}


def _build_nc():
    f32 = mybir.dt.float32
    nc = bacc.Bacc(
        "TRN2",
        target_bir_lowering=False,
        debug=False,
        enable_asserts=False,
        num_devices=NCORES,
    )
    xs = nc.dram_tensor("xs", [ROWS, F], f32, kind="ExternalInput")
    xps = nc.dram_tensor("xps", [ROWS, F], f32, kind="ExternalInput")
    wc = nc.dram_tensor("wc", [P, NT], f32, kind="ExternalInput")
    owc = nc.dram_tensor("owc", [P, NT], f32, kind="ExternalInput")
    ys = nc.dram_tensor("ys", [SHARD, NCLS], f32, kind="ExternalInput")
    yps = nc.dram_tensor("yps", [SHARD, NCLS], f32, kind="ExternalInput")
    wy = nc.dram_tensor("wy", [SHARD, 2], f32, kind="ExternalInput")
    ox = nc.dram_tensor("ox", [ROWS, F], f32, kind="ExternalOutput")
    oy = nc.dram_tensor("oy", [SHARD, NCLS], f32, kind="ExternalOutput")

    Copy = mybir.ActivationFunctionType.Copy
    mult = mybir.AluOpType.mult
    add = mybir.AluOpType.add

    with TileContext(nc) as tc:
        with (
            tc.tile_pool(name="const", bufs=1) as cp,
            tc.tile_pool(name="io", bufs=3) as io,
            tc.tile_pool(name="aux", bufs=2) as aux,
        ):
            wct = cp.tile([P, NT], f32, tag="wct")
            owct = cp.tile([P, NT], f32, tag="owct")
            nc.sync.dma_start(out=wct[:], in_=wc[:])
            nc.sync.dma_start(out=owct[:], in_=owc[:])

            # y path: one tiny [32, 8] tile
            wyt = cp.tile([SHARD, 2], f32, tag="wyt")
            yt = cp.tile([SHARD, NCLS], f32, tag="yt")
            ypt = cp.tile([SHARD, NCLS], f32, tag="ypt")
            twy = cp.tile([SHARD, NCLS], f32, tag="twy")
            oyt = cp.tile([SHARD, NCLS], f32, tag="oyt")
            nc.sync.dma_start(out=wyt[:], in_=wy[:])
            nc.sync.dma_start(out=yt[:], in_=ys[:])
            nc.sync.dma_start(out=ypt[:], in_=yps[:])
            nc.scalar.activation(twy[:], yt[:], Copy, scale=wyt[:, 0:1])
            nc.vector.scalar_tensor_tensor(
                oyt[:], ypt[:], wyt[:, 1:2], twy[:], mult, add
            )
            nc.sync.dma_start(out=oy[:], in_=oyt[:])

            for t in range(NT):
                xt = io.tile([P, F], f32, tag="x")
                xpt = io.tile([P, F], f32, tag="xp")
                nc.sync.dma_start(out=xt[:], in_=xs[t * P : (t + 1) * P, :])
                nc.sync.dma_start(out=xpt[:], in_=xps[t * P : (t + 1) * P, :])
                tw = aux.tile([P, F], f32, tag="tw")
                nc.scalar.activation(tw[:], xt[:], Copy, scale=wct[:, t : t + 1])
                ot = io.tile([P, F], f32, tag="o")
                nc.vector.scalar_tensor_tensor(
                    ot[:], xpt[:], owct[:, t : t + 1], tw[:], mult, add
                )
                nc.sync.dma_start(out=ox[t * P : (t + 1) * P, :], in_=ot[:])

    nc.compile()
    return nc


def get_nc():
    if "nc" not in _CACHE:
        _CACHE["nc"] = _build_nc()
    return _CACHE["nc"]


def make_in_maps(x, y, perm, mask, lam):
    """Host-side sharding: slice N across cores and materialize the
    cross-shard gather x[perm] / y[perm] per shard."""
    x = np.ascontiguousarray(x, dtype=np.float32)
    y = np.ascontiguousarray(y, dtype=np.float32)
    perm = np.asarray(perm).astype(np.int64)
    mask_b = np.asarray(mask).astype(bool)
    lam = np.asarray(lam, dtype=np.float32)

    w = np.where(mask_b, lam, np.float32(1.0)).astype(np.float32)
    omw = (np.float32(1.0) - w).astype(np.float32)

    in_maps = []
    for k in range(NCORES):
        sl = slice(k * SHARD, (k + 1) * SHARD)
        psl = perm[sl]
        w_sh = w[sl]
        omw_sh = omw[sl]
        # wc[p, t] = w_sh[t*SPT + p // PPS]
        wc = np.ascontiguousarray(np.repeat(w_sh.reshape(NT, SPT), PPS, axis=1).T)
        owc = np.ascontiguousarray(np.repeat(omw_sh.reshape(NT, SPT), PPS, axis=1).T)
        wy = np.ascontiguousarray(np.stack([w_sh, omw_sh], axis=1))
        in_maps.append(
            {
                "xs": x[sl].reshape(ROWS, F),
                "xps": np.ascontiguousarray(x[psl].reshape(ROWS, F)),
                "wc": wc,
                "owc": owc,
                "ys": np.ascontiguousarray(y[sl]),
                "yps": np.ascontiguousarray(y[psl]),
                "wy": wy,
            }
        )
    return in_maps


def assemble(results):
    aug_x = np.empty((N, C, L), np.float32)
    aug_y = np.empty((N, NCLS), np.float32)
    for k in range(NCORES):
        sl = slice(k * SHARD, (k + 1) * SHARD)
        aug_x[sl] = np.asarray(results[k]["ox"]).reshape(SHARD, C, L)
        aug_y[sl] = np.asarray(results[k]["oy"])
    return aug_x, aug_y


def kernel(x, y, perm, mask, lam):
    nc = get_nc()
    in_maps = make_in_maps(x, y, perm, mask, lam)
    res = run_bass_kernel_spmd(nc, in_maps, core_ids=list(range(NCORES)))
    return assemble(res.results)


# revision 3
# speedup vs baseline: 1.2514x; 1.2514x over previous
"""MixUp1D on Trainium2 (Bass/Tile), 8-core data-parallel.

Computes, for x:(N,C,L), y:(N,NC), perm/mask/lam:(N,):
    w        = where(mask, lam, 1.0)                  (host, O(N) scalars)
    aug_x[i] = w[i]*x[i] + (1-w[i])*x[perm[i]]
    aug_y[i] = w[i]*y[i] + (1-w[i])*y[perm[i]]

Sharding: N is split across 8 cores (32 samples each). perm indexes the
whole batch, so the cross-shard gather x[perm] is materialized on the
host while slicing the per-core inputs (the "all-to-all" realized at
shard time); each core then streams its 32 (x, x[perm]) sample pairs
and blends them with per-partition scalar weights.

Per-core device layout: the 32-sample shard (32 x 262144 f32) is viewed
as (2048, 4096); each [128, 4096] tile is one contiguous 2 MB DMA and
holds exactly 2 samples (64 partitions each), so the per-sample weight
becomes a per-partition scalar:
    tw = ACT:  w * x            (activation Copy with per-partition scale)
    o  = DVE:  (xp * (1-w)) + tw  (fused scalar_tensor_tensor)
"""

import numpy as np

import concourse.bacc as bacc
import concourse.mybir as mybir
from concourse.tile import TileContext
from concourse.bass_utils import run_bass_kernel_spmd

# Problem shapes (fixed for this problem)
N, C, L = 256, 16, 16384
NCLS = 8
NCORES = 8
SHARD = N // NCORES          # 32 samples per core
ELEM = C * L                 # 262144 elements per sample
F = 8192                     # SBUF tile free dim
ROWS = SHARD * ELEM // F     # 1024 rows of F per core
P = 128                      # SBUF partitions
NT = ROWS // P               # 8 x-tiles per core
SPT = (P * F) // ELEM        # samples per tile (4)
PPS = P // SPT               # partitions per sample (32)

_CACHE: dict = {}


def _build_nc():
    f32 = mybir.dt.float32
    nc = bacc.Bacc(
        "TRN2",
        target_bir_lowering=False,
        debug=False,
        enable_asserts=False,
        num_devices=NCORES,
    )
    xs = nc.dram_tensor("xs", [ROWS, F], f32, kind="ExternalInput")
    xps = nc.dram_tensor("xps", [ROWS, F], f32, kind="ExternalInput")
    wc = nc.dram_tensor("wc", [P, NT], f32, kind="ExternalInput")
    owc = nc.dram_tensor("owc", [P, NT], f32, kind="ExternalInput")
    ys = nc.dram_tensor("ys", [SHARD, NCLS], f32, kind="ExternalInput")
    yps = nc.dram_tensor("yps", [SHARD, NCLS], f32, kind="ExternalInput")
    wy = nc.dram_tensor("wy", [SHARD, 2], f32, kind="ExternalInput")
    ox = nc.dram_tensor("ox", [ROWS, F], f32, kind="ExternalOutput")
    oy = nc.dram_tensor("oy", [SHARD, NCLS], f32, kind="ExternalOutput")

    Copy = mybir.ActivationFunctionType.Copy
    mult = mybir.AluOpType.mult
    add = mybir.AluOpType.add

    with TileContext(nc) as tc:
        with (
            tc.tile_pool(name="const", bufs=1) as cp,
            tc.tile_pool(name="io", bufs=3) as io,
        ):
            wct = cp.tile([P, NT], f32, tag="wct")
            owct = cp.tile([P, NT], f32, tag="owct")
            nc.sync.dma_start(out=wct[:], in_=wc[:])
            nc.sync.dma_start(out=owct[:], in_=owc[:])

            for t in range(NT):
                xt = io.tile([P, F], f32, tag="x")
                xpt = io.tile([P, F], f32, tag="xp")
                nc.sync.dma_start(out=xt[:], in_=xs[t * P : (t + 1) * P, :])
                nc.sync.dma_start(out=xpt[:], in_=xps[t * P : (t + 1) * P, :])
                # x_t *= w (in-place on ScalarE)
                nc.scalar.activation(xt[:], xt[:], Copy, scale=wct[:, t : t + 1])
                # xp_t = (xp_t * (1-w)) + x_t (fused, in-place on VectorE)
                nc.vector.scalar_tensor_tensor(
                    xpt[:], xpt[:], owct[:, t : t + 1], xt[:], mult, add
                )
                # store from the ACT HWDGE ring so loads (SP ring) aren't
                # head-of-line blocked behind stores
                nc.scalar.dma_start(out=ox[t * P : (t + 1) * P, :], in_=xpt[:])

            # y path: one tiny [32, 8] tile, placed in the x-loop's tail
            wyt = cp.tile([SHARD, 2], f32, tag="wyt")
            yt = cp.tile([SHARD, NCLS], f32, tag="yt")
            ypt = cp.tile([SHARD, NCLS], f32, tag="ypt")
            nc.sync.dma_start(out=wyt[:], in_=wy[:])
            nc.sync.dma_start(out=yt[:], in_=ys[:])
            nc.sync.dma_start(out=ypt[:], in_=yps[:])
            nc.scalar.activation(yt[:], yt[:], Copy, scale=wyt[:, 0:1])
            nc.vector.scalar_tensor_tensor(
                ypt[:], ypt[:], wyt[:, 1:2], yt[:], mult, add
            )
            nc.scalar.dma_start(out=oy[:], in_=ypt[:])

    nc.compile()
    return nc


def get_nc():
    if "nc" not in _CACHE:
        _CACHE["nc"] = _build_nc()
    return _CACHE["nc"]


def make_in_maps(x, y, perm, mask, lam):
    """Host-side sharding: slice N across cores and materialize the
    cross-shard gather x[perm] / y[perm] per shard."""
    x = np.ascontiguousarray(x, dtype=np.float32)
    y = np.ascontiguousarray(y, dtype=np.float32)
    perm = np.asarray(perm).astype(np.int64)
    mask_b = np.asarray(mask).astype(bool)
    lam = np.asarray(lam, dtype=np.float32)

    w = np.where(mask_b, lam, np.float32(1.0)).astype(np.float32)
    omw = (np.float32(1.0) - w).astype(np.float32)

    in_maps = []
    for k in range(NCORES):
        sl = slice(k * SHARD, (k + 1) * SHARD)
        psl = perm[sl]
        w_sh = w[sl]
        omw_sh = omw[sl]
        # wc[p, t] = w_sh[t*SPT + p // PPS]
        wc = np.ascontiguousarray(np.repeat(w_sh.reshape(NT, SPT), PPS, axis=1).T)
        owc = np.ascontiguousarray(np.repeat(omw_sh.reshape(NT, SPT), PPS, axis=1).T)
        wy = np.ascontiguousarray(np.stack([w_sh, omw_sh], axis=1))
        in_maps.append(
            {
                "xs": x[sl].reshape(ROWS, F),
                "xps": np.ascontiguousarray(x[psl].reshape(ROWS, F)),
                "wc": wc,
                "owc": owc,
                "ys": np.ascontiguousarray(y[sl]),
                "yps": np.ascontiguousarray(y[psl]),
                "wy": wy,
            }
        )
    return in_maps


def assemble(results):
    aug_x = np.empty((N, C, L), np.float32)
    aug_y = np.empty((N, NCLS), np.float32)
    for k in range(NCORES):
        sl = slice(k * SHARD, (k + 1) * SHARD)
        aug_x[sl] = np.asarray(results[k]["ox"]).reshape(SHARD, C, L)
        aug_y[sl] = np.asarray(results[k]["oy"])
    return aug_x, aug_y


def kernel(x, y, perm, mask, lam):
    nc = get_nc()
    in_maps = make_in_maps(x, y, perm, mask, lam)
    res = run_bass_kernel_spmd(nc, in_maps, core_ids=list(range(NCORES)))
    return assemble(res.results)


# revision 4
# speedup vs baseline: 1.4047x; 1.1225x over previous
"""MixUp1D on Trainium2 (Bass/Tile), 8-core data-parallel.

Computes, for x:(N,C,L), y:(N,NC), perm/mask/lam:(N,):
    w        = where(mask, lam, 1.0)                  (host, O(N) scalars)
    aug_x[i] = w[i]*x[i] + (1-w[i])*x[perm[i]]
    aug_y[i] = w[i]*y[i] + (1-w[i])*y[perm[i]]

Sharding: N is split across 8 cores (32 samples each). perm indexes the
whole batch, so the cross-shard gather x[perm] is materialized on the
host while slicing the per-core inputs (the "all-to-all" realized at
shard time).

Per-core device layout: each 4 MB [128, 8192] SBUF tile is one
contiguous DMA holding exactly 4 samples (32 partitions each), so the
per-sample weight becomes a per-partition scalar:
    x_t  *= w                 (activation Copy, per-partition scale, in place)
    xp_t  = xp_t*(1-w) + x_t  (fused scalar_tensor_tensor, in place)

Mask elision: samples with mask=False don't need x[perm] (out == x
exactly, since w=1 makes the blend 1*x + 0*xp). The host reorders each
shard masked-first; the kernel blends only the first 4*NB samples
(NB = max masked count over cores, rounded up to a whole tile) and
emits plain DRAM->DRAM copies for the rest, skipping their x[perm]
read. NB=8 degenerates to blend-everything. One NEFF per NB, cached.
"""

import math

import numpy as np

import concourse.bacc as bacc
import concourse.mybir as mybir
from concourse.tile import TileContext
from concourse.bass_utils import run_bass_kernel_spmd

# Problem shapes (fixed for this problem)
N, C, L = 256, 16, 16384
NCLS = 8
NCORES = 8
SHARD = N // NCORES          # 32 samples per core
ELEM = C * L                 # 262144 elements per sample
F = 8192                     # SBUF tile free dim
ROWS = SHARD * ELEM // F     # 1024 rows of F per core
P = 128                      # SBUF partitions
NTILES = ROWS // P           # 8 tiles per core
SPT = (P * F) // ELEM        # samples per tile (4)
PPS = P // SPT               # partitions per sample (32)

_CACHE: dict = {}


def _build_nc(nb: int):
    """Build + compile the SPMD program with `nb` blend tiles and
    `NTILES - nb` pure-copy tiles (1 <= nb <= NTILES)."""
    f32 = mybir.dt.float32
    nc = bacc.Bacc(
        "TRN2",
        target_bir_lowering=False,
        debug=False,
        enable_asserts=False,
        num_devices=NCORES,
    )
    xs = nc.dram_tensor("xs", [ROWS, F], f32, kind="ExternalInput")
    xps = nc.dram_tensor("xps", [nb * P, F], f32, kind="ExternalInput")
    wc = nc.dram_tensor("wc", [P, nb], f32, kind="ExternalInput")
    owc = nc.dram_tensor("owc", [P, nb], f32, kind="ExternalInput")
    ys = nc.dram_tensor("ys", [SHARD, NCLS], f32, kind="ExternalInput")
    yps = nc.dram_tensor("yps", [SHARD, NCLS], f32, kind="ExternalInput")
    wy = nc.dram_tensor("wy", [SHARD, 2], f32, kind="ExternalInput")
    ox = nc.dram_tensor("ox", [ROWS, F], f32, kind="ExternalOutput")
    oy = nc.dram_tensor("oy", [SHARD, NCLS], f32, kind="ExternalOutput")

    Copy = mybir.ActivationFunctionType.Copy
    mult = mybir.AluOpType.mult
    add = mybir.AluOpType.add

    with TileContext(nc) as tc:
        with (
            tc.tile_pool(name="const", bufs=1) as cp,
            tc.tile_pool(name="io", bufs=3) as io,
        ):
            wct = cp.tile([P, nb], f32, tag="wct")
            owct = cp.tile([P, nb], f32, tag="owct")
            nc.sync.dma_start(out=wct[:], in_=wc[:])
            nc.sync.dma_start(out=owct[:], in_=owc[:])

            # Unmasked tail: out == x, no x[perm] read needed.
            # Independent DRAM->DRAM copies on the SWDGE ring.
            for t in range(nb, NTILES):
                nc.gpsimd.dma_start(
                    out=ox[t * P : (t + 1) * P, :], in_=xs[t * P : (t + 1) * P, :]
                )

            for t in range(nb):
                xt = io.tile([P, F], f32, tag="x")
                xpt = io.tile([P, F], f32, tag="xp")
                nc.sync.dma_start(out=xt[:], in_=xs[t * P : (t + 1) * P, :])
                nc.sync.dma_start(out=xpt[:], in_=xps[t * P : (t + 1) * P, :])
                # x_t *= w (in-place on ScalarE)
                nc.scalar.activation(xt[:], xt[:], Copy, scale=wct[:, t : t + 1])
                # xp_t = (xp_t * (1-w)) + x_t (fused, in-place on VectorE)
                nc.vector.scalar_tensor_tensor(
                    xpt[:], xpt[:], owct[:, t : t + 1], xt[:], mult, add
                )
                # store from the ACT HWDGE ring so loads (SP ring) aren't
                # head-of-line blocked behind stores
                nc.scalar.dma_start(out=ox[t * P : (t + 1) * P, :], in_=xpt[:])

            # y path: one tiny [32, 8] tile (full blend, original order)
            wyt = cp.tile([SHARD, 2], f32, tag="wyt")
            yt = cp.tile([SHARD, NCLS], f32, tag="yt")
            ypt = cp.tile([SHARD, NCLS], f32, tag="ypt")
            nc.sync.dma_start(out=wyt[:], in_=wy[:])
            nc.sync.dma_start(out=yt[:], in_=ys[:])
            nc.sync.dma_start(out=ypt[:], in_=yps[:])
            nc.scalar.activation(yt[:], yt[:], Copy, scale=wyt[:, 0:1])
            nc.vector.scalar_tensor_tensor(
                ypt[:], ypt[:], wyt[:, 1:2], yt[:], mult, add
            )
            nc.scalar.dma_start(out=oy[:], in_=ypt[:])

    nc.compile()
    return nc


def get_nc(nb: int = NTILES):
    if nb not in _CACHE:
        _CACHE[nb] = _build_nc(nb)
    return _CACHE[nb]


def _plan(mask_b):
    """Pick the blend-tile count and per-core masked-first sample order."""
    orders = []
    max_masked = 0
    for k in range(NCORES):
        m = mask_b[k * SHARD : (k + 1) * SHARD]
        cnt = int(m.sum())
        max_masked = max(max_masked, cnt)
        orders.append(np.concatenate([np.flatnonzero(m), np.flatnonzero(~m)]))
    nb = min(NTILES, max(1, math.ceil(max_masked / SPT)))
    return nb, orders


def make_in_maps(x, y, perm, mask, lam):
    """Host-side sharding: slice N across cores, reorder masked-first,
    and materialize the cross-shard gather x[perm] for blend rows only."""
    x = np.ascontiguousarray(x, dtype=np.float32)
    y = np.ascontiguousarray(y, dtype=np.float32)
    perm = np.asarray(perm).astype(np.int64)
    mask_b = np.asarray(mask).astype(bool)
    lam = np.asarray(lam, dtype=np.float32)

    w = np.where(mask_b, lam, np.float32(1.0)).astype(np.float32)
    omw = (np.float32(1.0) - w).astype(np.float32)

    nb, orders = _plan(mask_b)
    blend = nb * SPT  # samples blended per core

    in_maps = []
    for k in range(NCORES):
        sl = slice(k * SHARD, (k + 1) * SHARD)
        order = orders[k]
        gidx = k * SHARD + order            # global sample ids, masked first
        w_r = w[gidx]
        omw_r = omw[gidx]
        psl = perm[gidx[:blend]]            # partner rows for blend region only
        wc = np.ascontiguousarray(
            np.repeat(w_r[:blend].reshape(nb, SPT), PPS, axis=1).T
        )
        owc = np.ascontiguousarray(
            np.repeat(omw_r[:blend].reshape(nb, SPT), PPS, axis=1).T
        )
        wy = np.ascontiguousarray(np.stack([w[sl], omw[sl]], axis=1))
        in_maps.append(
            {
                "xs": np.ascontiguousarray(x[gidx].reshape(ROWS, F)),
                "xps": np.ascontiguousarray(x[psl].reshape(nb * P, F)),
                "wc": wc,
                "owc": owc,
                "ys": np.ascontiguousarray(y[sl]),
                "yps": np.ascontiguousarray(y[perm[sl]]),
                "wy": wy,
            }
        )
    return nb, orders, in_maps


def assemble(results, orders):
    aug_x = np.empty((N, C, L), np.float32)
    aug_y = np.empty((N, NCLS), np.float32)
    for k in range(NCORES):
        sl = slice(k * SHARD, (k + 1) * SHARD)
        gidx = k * SHARD + orders[k]
        aug_x[gidx] = np.asarray(results[k]["ox"]).reshape(SHARD, C, L)
        aug_y[sl] = np.asarray(results[k]["oy"])
    return aug_x, aug_y


def kernel(x, y, perm, mask, lam):
    nb, orders, in_maps = make_in_maps(x, y, perm, mask, lam)
    nc = get_nc(nb)
    res = run_bass_kernel_spmd(nc, in_maps, core_ids=list(range(NCORES)))
    return assemble(res.results, orders)
